# revision 19
# baseline (speedup 1.0000x reference)
"""Causal multi-head attention block (QKV proj -> causal attention -> out proj)
for Trainium2, distributed over 8 NeuronCores.

Sharding: core c handles batch b = c//2 and head-group g = c%2 (8 of 16 heads).
Each core computes qkv for its group's columns of w_attn, runs causal attention
for its 8 heads, and multiplies by its group's rows of w_proj, producing a
partial y[b]. The host sums the two partials per batch and adds b_proj.

All on-chip tensors and DMAs are bf16 (PSUM accumulation stays fp32); the
2e-2 rel-err budget dwarfs bf16's ~3e-3. The kernel works in transposed
layouts end-to-end (host passes x[b].T as bf16, device returns y[b].T f32):
  q^T,k^T = w_{q,k}^T-chunks @ x^T      [cols, tok]
  s^T     = k_h^T-chunks    @ q_h^T     [k_tok, q_tok]
Attention runs per HEAD PAIR: heads 2hp (SBUF partitions 0:64) and 2hp+1
(64:128) issue their K=64 score matmuls back-to-back as PE row-tiles
T0/T8 (tile_position auto-derives from base partitions), which the PE
array executes concurrently. Both heads' score blocks land in one 2-bank
PSUM tile, so a single exp (ACT) and a single tri-mask multiply cover the
pair. v is padded with 64 ones columns per head, so the pv matmul lands
softmax denominators replicated on psum partitions 64..127; normalization
is one fast-approx reciprocal + one multiply per (head, half) on DVE.
  y^T     = w_proj-chunks   @ out_norm^T
Emission order pipelines phases: chunked input DMAs, v first, then per
head-pair q/k projection + qc=0 attention, then qc=1 attention with the
token-half-0 output projection interleaved, then the token-half-1
projection tail with evictions alternating ACT/DVE.
"""

import math
import sys

import numpy as np

if "/opt/trn_rl_repo" not in sys.path:
    sys.path.insert(0, "/opt/trn_rl_repo")

B, S, D = 4, 1024, 1024
H = 16
HPG = 8              # heads per group (2 groups of 8)
hd = D // H          # 64
GC = HPG * hd        # 512 cols per group for each of q,k,v
P = 128
DC = D // P          # 8 contraction chunks

_CACHE = {}


def _build(repeat=1, mask_pool=True, qkt_act=False, blk_batch=2, in_act=0,
           fill=3, pre_fill=2, safe_recip=True, pool_mults=False):
    from collections import deque

    import concourse.mybir as mybir
    import concourse.tile as tile
    from concourse import bacc
    from concourse.masks import make_upper_triangular

    f32 = mybir.dt.float32
    bf16 = mybir.dt.bfloat16
    Exp = mybir.ActivationFunctionType.Exp
    Ln = mybir.ActivationFunctionType.Ln
    mult = mybir.AluOpType.mult

    nc = bacc.Bacc("TRN2", target_bir_lowering=False, debug=False, num_devices=8)
    xT = nc.dram_tensor("xT", [D, S], bf16, kind="ExternalInput").ap()
    wq = nc.dram_tensor("wq", [D, GC], bf16, kind="ExternalInput").ap()
    wk = nc.dram_tensor("wk", [D, GC], bf16, kind="ExternalInput").ap()
    wv = nc.dram_tensor("wv", [D, GC], bf16, kind="ExternalInput").ap()
    wp = nc.dram_tensor("wp", [GC, D], bf16, kind="ExternalInput").ap()
    yT = nc.dram_tensor("yT", [D, S], bf16, kind="ExternalOutput").ap()

    scale = 1.0 / math.sqrt(hd)

    with tile.TileContext(nc) as tc:
        with tc.tile_pool(name="const", bufs=1) as const, \
             tc.tile_pool(name="big", bufs=1) as big, \
             tc.tile_pool(name="pt", bufs=6) as ptp, \
             tc.tile_pool(name="small", bufs=4) as small, \
             tc.tile_pool(name="yt", bufs=4) as ytp, \
             tc.tile_pool(name="ps", bufs=1, space="PSUM") as ps:

          tri_f = const.tile([P, P], f32, tag="tri_f")  # keep iff k <= q
          make_upper_triangular(nc, tri_f[:], val=1.0, diag=True)
          tri = const.tile([P, P], bf16, tag="tri")
          nc.vector.tensor_copy(out=tri[:], in_=tri_f[:])
          ones_col = const.tile([P, 1], bf16, tag="ones_col")
          nc.any.memset(ones_col[:], 1.0)
          # v padded with 64 ones columns per head: the pv matmul then
          # lands sum(p) replicated on psum partitions 64..127, so softmax
          # normalization needs no partition broadcast at all. The ones
          # region is written once; v_group only overwrites [..., 0:hd].
          vaug = const.tile([P, S // P, HPG, 2 * hd], bf16, tag="vaug")
          nc.vector.tensor_copy(
              out=vaug[:, :, :, hd:2 * hd],
              in_=ones_col[:].to_broadcast([P, S // P, HPG, hd]))

          for _rep in range(repeat):
            # chunked input loads so compute can start on early chunks;
            # bufs=2 lets the next iteration's loads stream while this
            # iteration still reads the previous buffer.
            xt = big.tile([P, DC, S], bf16, tag="xt", bufs=2)
            xTr = xT.rearrange("(dc p) t -> p dc t", p=P)
            wqt = big.tile([P, DC, GC], bf16, tag="wq", bufs=2)
            wqr = wq.rearrange("(dc p) c -> p dc c", p=P)
            wkt = big.tile([P, DC, GC], bf16, tag="wk", bufs=2)
            wkr = wk.rearrange("(dc p) c -> p dc c", p=P)
            wvt = big.tile([P, DC, GC], bf16, tag="wv", bufs=2)
            wvr = wv.rearrange("(dc p) c -> p dc c", p=P)
            # input loads optionally on the Activation hwdge queue: they are
            # fenced by their consumers' semaphores (no exit-fence needed),
            # and a separate queue lets the next iteration's inputs stream
            # while this iteration's outputs drain on the sync queue.
            inq = nc.scalar if in_act else nc.sync
            for dc in range(DC):
                inq.dma_start(xt[:, dc:dc + 1, :], xTr[:, dc:dc + 1, :])
                inq.dma_start(wvt[:, dc:dc + 1, :], wvr[:, dc:dc + 1, :])
            for dc in range(DC):
                inq.dma_start(wqt[:, dc:dc + 1, :], wqr[:, dc:dc + 1, :])
                inq.dma_start(wkt[:, dc:dc + 1, :], wkr[:, dc:dc + 1, :])
            wpt = big.tile([P, GC // P, D], bf16, tag="wp", bufs=2)
            wpr = wp.rearrange("(cc p) o -> p cc o", p=P)
            for cc in range(GC // P):
                inq.dma_start(wpt[:, cc:cc + 1, :], wpr[:, cc:cc + 1, :])

            # q^T/k^T for the group: [col(128), chunk, tok]; chunks 0-3 = q,
            # 4-7 = k; head pair hp lives in chunk hp (2hp on partitions 0:64,
            # 2hp+1 on 64:128)
            qkt = big.tile([P, 2 * GC // P, S], bf16, tag="qkt")
            # normalized attention output ^T: [chan(128), chan_chunk, tok]
            outt = big.tile([P, GC // P, S], bf16, tag="outt")

            # ---- projection groups as generators: one PE matmul per
            # next(), eviction emitted with the final step. Used directly
            # (run to exhaustion) or as fillers interleaved into the
            # attention pipeline to keep the PE streaming while ACT/Pool
            # work through the exp->mask chain.
            def qk_steps(cc8, t5):
                src = wqt if cc8 < 4 else wkt
                cbase = (cc8 % 4) * P
                acc = ps.tile([P, 512], f32, tag="gacc", bufs=2)
                for dc in range(DC):
                    nc.tensor.matmul(
                        acc[:],
                        src[:, dc, cbase:cbase + P],
                        xt[:, dc, t5 * 512:(t5 + 1) * 512],
                        start=(dc == 0), stop=(dc == DC - 1),
                    )
                    if dc == DC - 1:
                        if qkt_act:
                            nc.scalar.copy(
                                qkt[:, cc8, t5 * 512:(t5 + 1) * 512], acc[:])
                        else:
                            nc.vector.tensor_copy(
                                out=qkt[:, cc8, t5 * 512:(t5 + 1) * 512],
                                in_=acc[:])
                    yield

            def v_steps(t8, evict="dve"):
                acc = ps.tile([P, 512], f32, tag="gacc", bufs=2)
                for dc in range(DC):
                    nc.tensor.matmul(
                        acc[:],
                        xt[:, dc, t8 * P:(t8 + 1) * P],
                        wvt[:, dc, :],
                        start=(dc == 0), stop=(dc == DC - 1),
                    )
                    if dc == DC - 1:
                        # filler v groups evict on ACT so the DVE queue's
                        # qkt evictions (which gate the next pair's scores)
                        # aren't delayed behind them (Pool cannot read
                        # PSUM on real hardware)
                        if evict == "act":
                            nc.scalar.copy(
                                vaug[:, t8, :, 0:hd],
                                acc[:].rearrange("p (h j) -> p h j", h=HPG))
                        else:
                            nc.vector.tensor_copy(
                                out=vaug[:, t8, :, 0:hd],
                                in_=acc[:].rearrange("p (h j) -> p h j",
                                                     h=HPG))
                    yield

            def proj_steps(t5, oc, evict="dve"):
                acc = ps.tile([P, 512], f32, tag="gacc", bufs=2)
                for cc in range(GC // P):
                    nc.tensor.matmul(
                        acc[:],
                        wpt[:, cc, oc * P:(oc + 1) * P],
                        outt[:, cc, t5 * 512:(t5 + 1) * 512],
                        start=(cc == 0), stop=(cc == GC // P - 1),
                    )
                    if cc == GC // P - 1:
                        # groups interleaved with attention evict on ACT so
                        # the DVE norm chains aren't queued behind them;
                        # the tail alternates ACT/DVE
                        yt = ytp.tile([P, 512], bf16, tag="yt")
                        if evict == "act":
                            nc.scalar.copy(yt[:], acc[:])
                        else:
                            nc.vector.tensor_copy(out=yt[:], in_=acc[:])
                        nc.sync.dma_start(
                            yT[oc * P:(oc + 1) * P,
                               t5 * 512:(t5 + 1) * 512], yt[:])
                    yield

            class _Fillers:
                """FIFO of group generators; take(n) emits up to n PE
                matmuls by advancing the head generator. A partially
                advanced group holds a gacc PSUM ring slot, so direct
                (non-filler) gacc groups must finish_open() first or the
                2-slot ring can wrap onto the open group and deadlock the
                in-order PE queue."""

                def __init__(self):
                    self.q = deque()
                    self.open = False

                def add(self, gen):
                    self.q.append(gen)
                    return gen

                def take(self, n):
                    while n > 0 and self.q:
                        try:
                            next(self.q[0])
                            self.open = True
                            n -= 1
                        except StopIteration:
                            self.q.popleft()
                            self.open = False

                def finish_open(self):
                    if self.q and self.open:
                        g = self.q.popleft()
                        for _ in g:
                            pass
                        self.open = False

                def finish_through(self, gen):
                    # advance the FIFO until `gen` has fully emitted: a
                    # consumer is about to be emitted whose semaphores only
                    # exist if the producer's instructions precede it
                    while gen in self.q:
                        try:
                            next(self.q[0])
                            self.open = True
                        except StopIteration:
                            self.q.popleft()
                            self.open = False

                def drain(self):
                    self.finish_open()
                    while self.q:
                        g = self.q.popleft()
                        for _ in g:
                            pass

            fl = _Fillers()

            def run_now(gen):
                fl.finish_open()
                for _ in gen:
                    pass

            # ---- causal attention, head-pair processed, transposed ----
            def attn_pair(hp, qc, norm_pool=False):
                qA = qkt[0:hd, hp, :]
                qB = qkt[hd:P, hp, :]
                kA = qkt[0:hd, 4 + hp, :]
                kB = qkt[hd:P, 4 + hp, :]
                accp = ps.tile([P, 2, 512], f32, tag="accp", bufs=1)
                # qc=0: [0..3] (kb=0 is the full-width diagonal block);
                # qc=1: full-width below-diagonal blocks first, then the
                # diagonal / partial-width blocks
                order = list(range(4 * qc)) + [4 * qc + p for p in range(4)]
                nblk = len(order)
                batches = [order[i:i + blk_batch]
                           for i in range(0, nblk, blk_batch)]

                def scores(batch):
                    res = []
                    for kb in batch:
                        p_off = kb - 4 * qc    # >=0 on diagonal blocks
                        start_col = max(0, p_off) * P
                        width = 512 - start_col
                        sp2 = ps.tile([P, 2, 512], f32, tag="sp2", bufs=2)
                        nc.tensor.matmul(
                            sp2[0:P, 0, 0:width],
                            kA[:, kb * P:(kb + 1) * P],
                            qA[:, qc * 512 + start_col:(qc + 1) * 512],
                            start=True, stop=True,
                        )
                        nc.tensor.matmul(
                            sp2[0:P, 1, 0:width],
                            kB[:, kb * P:(kb + 1) * P],
                            qB[:, qc * 512 + start_col:(qc + 1) * 512],
                            start=True, stop=True,
                        )
                        res.append((kb, start_col, width, sp2))
                    return res

                def expmask(sps):
                    res = []
                    for kb, start_col, width, sp2 in sps:
                        pt2 = ptp.tile([P, 2, 512], bf16, tag="pt2")
                        nc.scalar.activation(pt2[:, :, 0:width],
                                             sp2[:, :, 0:width],
                                             Exp, scale=scale)
                        if kb - 4 * qc >= 0:   # triangular mask part
                            eng = nc.gpsimd if mask_pool else nc.vector
                            eng.tensor_tensor(
                                pt2[:, :, 0:P], pt2[:, :, 0:P],
                                tri[:].rearrange("p (o k) -> p o k", o=1)
                                      .to_broadcast([P, 2, P]), mult)
                        res.append((kb, start_col, width, pt2))
                    return res

                def pv(pts, i0):
                    for j, (kb, start_col, width, pt2) in enumerate(pts):
                        i = i0 + j
                        nc.tensor.matmul(
                            accp[:, 0, start_col:512],
                            vaug[:, kb, 2 * hp, :],
                            pt2[:, 0, 0:width],
                            start=(i == 0), stop=(i == nblk - 1),
                        )
                        nc.tensor.matmul(
                            accp[:, 1, start_col:512],
                            vaug[:, kb, 2 * hp + 1, :],
                            pt2[:, 1, 0:width],
                            start=(i == 0), stop=(i == nblk - 1),
                        )

                # lookahead-1 software pipeline: the PE queue runs
                # [s(b0) s(b1) fillers pv(b0) s(b2) fillers pv(b1) ...] so
                # the PE streams scores of batch j+1 (plus fillers) while
                # ACT/Pool run exp+mask of batch j. sp2's 2-slot ring
                # makes s(b_{j+1}) wait exactly until exp(b_j) has read.
                # Batches with masked (diagonal) blocks have the longer
                # exp->mask chain, so they pull more fillers.
                fl.take(pre_fill)
                sps = scores(batches[0])
                i0 = 0
                for bi in range(len(batches)):
                    pts = expmask(sps)
                    nxt = (scores(batches[bi + 1])
                           if bi + 1 < len(batches) else None)
                    masked = any(kb - 4 * qc >= 0 for kb, _, _, _ in pts)
                    fl.take(fill if masked else 1)
                    pv(pts, i0)
                    i0 += len(pts)
                    sps = nxt

                # normalize: psum rows 64..127 hold rowsum replicated 64x
                # (from vaug's ones block). ONE full-accp copy to SBUF --
                # same DVE time as copying only the sums (the free-dim size
                # per lane is identical) -- releases the PSUM accumulator
                # immediately, so the next pair's pv never waits on the
                # reciprocal+multiply tail. That tail (fast reciprocal on
                # DVE -- the custom op misreads PSUM, hence SBUF staging --
                # and per-head multiplies on Pool/DVE) runs off the
                # critical path.
                accs = small.tile([P, 2, 512], f32, tag="accs", bufs=2)
                nc.vector.tensor_copy(out=accs[:], in_=accp[:])
                rsb = small.tile([hd, 2, 512], f32, tag="rsb")
                if safe_recip:
                    # stage the row-sums to a base-0 tile before the custom
                    # DVE op (off the critical path; accp already released)
                    rss = small.tile([hd, 2, 512], f32, tag="rss")
                    nc.vector.tensor_copy(out=rss[:], in_=accs[hd:P, :, :])
                    nc.vector.reciprocal_approx_fast(out=rsb[:], in_=rss[:])
                else:
                    nc.vector.reciprocal_approx_fast(out=rsb[:],
                                                     in_=accs[hd:P, :, :])
                meng = nc.gpsimd if norm_pool else nc.vector
                for h01 in (0, 1):
                    prow = hd * h01
                    meng.tensor_tensor(
                        outt[prow:prow + hd, hp, qc * 512:(qc + 1) * 512],
                        accs[0:hd, h01, :], rsb[:, h01, :], mult)

            # ---- emission schedule ----
            # The exp stream on ACT (~34us/iter) must be spread across the
            # whole program: every attention span needs enough independent
            # PE ballast alongside it. Phase 1 (qc=0, exp-light) keeps only
            # v(4..7) as fillers; the t5=1 q/k projections move into phase
            # 2 (qc=1, exp-heavy), staggered one pair ahead of their
            # consumer, together with the token-half-0 output projections.
            # proj(0)'s leftovers cover the final norm chains; the
            # token-half-1 projections close the iteration.
            for t8 in range(4):
                run_now(v_steps(t8))
            for hp in range(GC // P):
                run_now(qk_steps(hp, 0))
                run_now(qk_steps(4 + hp, 0))
                if hp == 0:
                    for t8 in range(4, S // P):
                        fl.add(v_steps(t8, evict="act"))
                attn_pair(hp, 0)
            fl.drain()
            run_now(qk_steps(0, 1))
            run_now(qk_steps(4, 1))
            qk_pend = {}
            for hp in range(GC // P):
                if hp + 1 < GC // P:
                    qk_pend[hp + 1] = (fl.add(qk_steps(hp + 1, 1)),
                                       fl.add(qk_steps(4 + hp + 1, 1)))
                fl.add(proj_steps(0, 2 * hp, evict="act"))
                if hp in qk_pend:
                    for g in qk_pend.pop(hp):
                        fl.finish_through(g)
                attn_pair(hp, 1, norm_pool=pool_mults)
            fl.drain()
            for oc in (1, 3, 5, 7):
                run_now(proj_steps(0, oc))
            for oc in range(D // P):
                run_now(proj_steps(1, oc, evict="act" if oc % 2 == 0
                                          else "dve"))

    nc.compile()
    return nc


def _get_nc(repeat=1, **kw):
    key = ("nc", repeat, tuple(sorted(kw.items())))
    if key not in _CACHE:
        _CACHE[key] = _build(repeat, **kw)
    return _CACHE[key]


def _bf16(a):
    from ml_dtypes import bfloat16
    return np.ascontiguousarray(a.astype(bfloat16))


def make_in_maps(x, w_attn, w_proj):
    """Per-core input shards (core c -> batch c//2, head-group c%2)."""
    in_maps = []
    xTs = [_bf16(x[b].T) for b in range(B)]
    wqs = [_bf16(w_attn[:, g * GC:(g + 1) * GC]) for g in range(2)]
    wks = [_bf16(w_attn[:, D + g * GC:D + (g + 1) * GC]) for g in range(2)]
    wvs = [_bf16(w_attn[:, 2 * D + g * GC:2 * D + (g + 1) * GC])
           for g in range(2)]
    wps = [_bf16(w_proj[g * GC:(g + 1) * GC, :]) for g in range(2)]
    for c in range(8):
        b, g = divmod(c, 2)
        in_maps.append({
            "xT": xTs[b],
            "wq": wqs[g],
            "wk": wks[g],
            "wv": wvs[g],
            "wp": wps[g],
        })
    return in_maps


def kernel(x, w_attn, b_attn, w_proj, b_proj):
    x = np.asarray(x, dtype=np.float32)
    w_attn = np.asarray(w_attn, dtype=np.float32)
    b_attn = np.asarray(b_attn, dtype=np.float32)
    w_proj = np.asarray(w_proj, dtype=np.float32)
    b_proj = np.asarray(b_proj, dtype=np.float32)

    if np.any(b_attn):
        # Spec guarantees b_attn == 0 (fill: zeros); exact fallback if not.
        return _numpy_reference(x, w_attn, b_attn, w_proj, b_proj)

    in_maps = make_in_maps(x, w_attn, w_proj)
    results = _run_cached(in_maps)
    y = np.empty((B, S, D), np.float32)
    for b in range(B):
        y[b] = (results[2 * b]["yT"].astype(np.float32).T
                + results[2 * b + 1]["yT"].astype(np.float32).T + b_proj)
    return y


def _run_cached(in_maps):
    """Execute the compiled module on 8 cores; the jitted PJRT runner is
    built once and reused so repeated kernel() calls skip retracing."""
    import jax
    from jax.sharding import Mesh, NamedSharding, PartitionSpec
    from jax.experimental.shard_map import shard_map
    import concourse.mybir as mybir
    from concourse.bass2jax import (_bass_exec_p, install_neuronx_cc_hook,
                                    partition_id_tensor)

    if "runner" not in _CACHE:
        install_neuronx_cc_hook()
        nc = _get_nc()
        partition_name = (nc.partition_id_tensor.name
                          if nc.partition_id_tensor else None)
        in_names, out_names, out_avals, zero_outs = [], [], [], []
        for alloc in nc.m.functions[0].allocations:
            if not isinstance(alloc, mybir.MemoryLocationSet):
                continue
            name = alloc.memorylocations[0].name
            if alloc.kind == "ExternalInput":
                if name != partition_name:
                    in_names.append(name)
            elif alloc.kind == "ExternalOutput":
                shape = tuple(alloc.tensor_shape)
                dtype = mybir.dt.np(alloc.dtype)
                out_names.append(name)
                out_avals.append(jax.core.ShapedArray(shape, dtype))
                zero_outs.append(np.zeros((8 * shape[0], *shape[1:]), dtype))
        all_in_names = list(in_names) + list(out_names)
        if partition_name is not None:
            all_in_names.append(partition_name)

        def _body(*args):
            operands = list(args)
            if partition_name is not None:
                operands.append(partition_id_tensor())
            return tuple(_bass_exec_p.bind(
                *operands,
                out_avals=tuple(out_avals),
                in_names=tuple(all_in_names),
                out_names=tuple(out_names),
                lowering_input_output_aliases=(),
                sim_require_finite=True,
                sim_require_nnan=True,
                nc=nc,
            ))

        devices = jax.devices()[:8]
        mesh = Mesh(np.asarray(devices), ("core",))
        n_ops = len(in_names) + len(out_names)
        fn = jax.jit(shard_map(
            _body, mesh=mesh,
            in_specs=(PartitionSpec("core"),) * n_ops,
            out_specs=(PartitionSpec("core"),) * len(out_names),
            check_rep=False), keep_unused=True)
        shard = NamedSharding(mesh, PartitionSpec("core"))
        zeros_dev = [jax.device_put(z, shard) for z in zero_outs]
        _CACHE["runner"] = (fn, in_names, out_names, zeros_dev, shard)

    fn, in_names, out_names, zeros_dev, shard = _CACHE["runner"]
    import jax
    concat_in = [np.concatenate([np.asarray(in_maps[c][n]) for c in range(8)],
                                axis=0) for n in in_names]
    dev_in = [jax.device_put(a, shard) for a in concat_in]
    out_arrs = fn(*dev_in, *zeros_dev)
    results = []
    for c in range(8):
        results.append({
            name: np.asarray(out_arrs[i]).reshape(8, -1, 1024)[c]
            for i, name in enumerate(out_names)})
    return results


def _numpy_reference(x, w_attn, b_attn, w_proj, b_proj):
    qkv = x @ w_attn + b_attn
    q, k, v = np.split(qkv, 3, axis=-1)

    def heads(t):
        return t.reshape(B, S, H, hd).transpose(0, 2, 1, 3)

    q, k, v = heads(q), heads(k), heads(v)
    scores = np.einsum("bhqd,bhkd->bhqk", q, k) / np.sqrt(np.float32(hd))
    causal = np.tril(np.ones((S, S), dtype=bool))[None, None]
    scores = np.where(causal, scores, -1e9)
    scores -= scores.max(axis=-1, keepdims=True)
    attn = np.exp(scores)
    attn /= attn.sum(axis=-1, keepdims=True)
    out = np.einsum("bhqk,bhkd->bhqd", attn, v)
    out = out.transpose(0, 2, 1, 3).reshape(B, S, D)
    return out @ w_proj + b_proj



# revision 20
# speedup vs baseline: 1.1524x; 1.1524x over previous
"""Causal multi-head attention block (QKV proj -> causal attention -> out proj)
for Trainium2, distributed over 8 NeuronCores.

Sharding: core c handles batch b = c//2 and head-group g = c%2 (8 of 16 heads).
Each core computes qkv for its group's columns of w_attn, runs causal attention
for its 8 heads, and multiplies by its group's rows of w_proj, producing a
partial y[b]. The host sums the two partials per batch and adds b_proj.

All on-chip tensors and DMAs are bf16 (PSUM accumulation stays fp32); the
2e-2 rel-err budget dwarfs bf16's ~3e-3. The kernel works in transposed
layouts end-to-end (host passes x[b].T as bf16, device returns y[b].T f32):
  q^T,k^T = w_{q,k}^T-chunks @ x^T      [cols, tok]
  s^T     = k_h^T-chunks    @ q_h^T     [k_tok, q_tok]
Attention runs per HEAD PAIR: heads 2hp (SBUF partitions 0:64) and 2hp+1
(64:128) issue their K=64 score matmuls back-to-back as PE row-tiles
T0/T8 (tile_position auto-derives from base partitions), which the PE
array executes concurrently. Both heads' score blocks land in one 2-bank
PSUM tile, so a single exp (ACT) and a single tri-mask multiply cover the
pair. v is padded with 64 ones columns per head, so the pv matmul lands
softmax denominators replicated on psum partitions 64..127; normalization
is one fast-approx reciprocal + one multiply per (head, half) on DVE.
  y^T     = w_proj-chunks   @ out_norm^T
Emission order pipelines phases: chunked input DMAs, v first, then per
head-pair q/k projection + qc=0 attention, then qc=1 attention with the
token-half-0 output projection interleaved, then the token-half-1
projection tail with evictions alternating ACT/DVE.
"""

import math
import sys

import numpy as np

if "/opt/trn_rl_repo" not in sys.path:
    sys.path.insert(0, "/opt/trn_rl_repo")

B, S, D = 4, 1024, 1024
H = 16
HPG = 8              # heads per group (2 groups of 8)
hd = D // H          # 64
GC = HPG * hd        # 512 cols per group for each of q,k,v
P = 128
DC = D // P          # 8 contraction chunks

_CACHE = {}


def _build(repeat=1, mask_pool=True, qkt_act=False, blk_batch=2, in_act=0,
           fill=3, pre_fill=2, safe_recip=True, pool_mults=True):
    from collections import deque

    import concourse.mybir as mybir
    import concourse.tile as tile
    from concourse import bacc
    from concourse.masks import make_upper_triangular

    f32 = mybir.dt.float32
    bf16 = mybir.dt.bfloat16
    Exp = mybir.ActivationFunctionType.Exp
    Ln = mybir.ActivationFunctionType.Ln
    mult = mybir.AluOpType.mult

    nc = bacc.Bacc("TRN2", target_bir_lowering=False, debug=False, num_devices=8)
    xT = nc.dram_tensor("xT", [D, S], bf16, kind="ExternalInput").ap()
    wq = nc.dram_tensor("wq", [D, GC], bf16, kind="ExternalInput").ap()
    wk = nc.dram_tensor("wk", [D, GC], bf16, kind="ExternalInput").ap()
    wv = nc.dram_tensor("wv", [D, GC], bf16, kind="ExternalInput").ap()
    wp = nc.dram_tensor("wp", [GC, D], bf16, kind="ExternalInput").ap()
    yT = nc.dram_tensor("yT", [D, S], bf16, kind="ExternalOutput").ap()

    scale = 1.0 / math.sqrt(hd)

    with tile.TileContext(nc) as tc:
        with tc.tile_pool(name="const", bufs=1) as const, \
             tc.tile_pool(name="big", bufs=1) as big, \
             tc.tile_pool(name="pt", bufs=6) as ptp, \
             tc.tile_pool(name="small", bufs=4) as small, \
             tc.tile_pool(name="yt", bufs=4) as ytp, \
             tc.tile_pool(name="ps", bufs=1, space="PSUM") as ps:

          tri_f = const.tile([P, P], f32, tag="tri_f")  # keep iff k <= q
          make_upper_triangular(nc, tri_f[:], val=1.0, diag=True)
          tri = const.tile([P, P], bf16, tag="tri")
          nc.vector.tensor_copy(out=tri[:], in_=tri_f[:])
          ones_col = const.tile([P, 1], bf16, tag="ones_col")
          nc.any.memset(ones_col[:], 1.0)
          # v padded with 64 ones columns per head: the pv matmul then
          # lands sum(p) replicated on psum partitions 64..127, so softmax
          # normalization needs no partition broadcast at all. The ones
          # region is written once; v_group only overwrites [..., 0:hd].
          vaug = const.tile([P, S // P, HPG, 2 * hd], bf16, tag="vaug")
          nc.vector.tensor_copy(
              out=vaug[:, :, :, hd:2 * hd],
              in_=ones_col[:].to_broadcast([P, S // P, HPG, hd]))

          for _rep in range(repeat):
            # chunked input loads so compute can start on early chunks;
            # bufs=2 lets the next iteration's loads stream while this
            # iteration still reads the previous buffer.
            xt = big.tile([P, DC, S], bf16, tag="xt", bufs=2)
            xTr = xT.rearrange("(dc p) t -> p dc t", p=P)
            wqt = big.tile([P, DC, GC], bf16, tag="wq", bufs=2)
            wqr = wq.rearrange("(dc p) c -> p dc c", p=P)
            wkt = big.tile([P, DC, GC], bf16, tag="wk", bufs=2)
            wkr = wk.rearrange("(dc p) c -> p dc c", p=P)
            wvt = big.tile([P, DC, GC], bf16, tag="wv", bufs=2)
            wvr = wv.rearrange("(dc p) c -> p dc c", p=P)
            # input loads optionally on the Activation hwdge queue: they are
            # fenced by their consumers' semaphores (no exit-fence needed),
            # and a separate queue lets the next iteration's inputs stream
            # while this iteration's outputs drain on the sync queue.
            inq = nc.scalar if in_act else nc.sync
            for dc in range(DC):
                inq.dma_start(xt[:, dc:dc + 1, :], xTr[:, dc:dc + 1, :])
                inq.dma_start(wvt[:, dc:dc + 1, :], wvr[:, dc:dc + 1, :])
            for dc in range(DC):
                inq.dma_start(wqt[:, dc:dc + 1, :], wqr[:, dc:dc + 1, :])
                inq.dma_start(wkt[:, dc:dc + 1, :], wkr[:, dc:dc + 1, :])
            wpt = big.tile([P, GC // P, D], bf16, tag="wp", bufs=2)
            wpr = wp.rearrange("(cc p) o -> p cc o", p=P)
            for cc in range(GC // P):
                inq.dma_start(wpt[:, cc:cc + 1, :], wpr[:, cc:cc + 1, :])

            # q^T/k^T for the group: [col(128), chunk, tok]; chunks 0-3 = q,
            # 4-7 = k; head pair hp lives in chunk hp (2hp on partitions 0:64,
            # 2hp+1 on 64:128)
            qkt = big.tile([P, 2 * GC // P, S], bf16, tag="qkt")
            # normalized attention output ^T: [chan(128), chan_chunk, tok]
            outt = big.tile([P, GC // P, S], bf16, tag="outt")

            # ---- projection groups as generators: one PE matmul per
            # next(), eviction emitted with the final step. Used directly
            # (run to exhaustion) or as fillers interleaved into the
            # attention pipeline to keep the PE streaming while ACT/Pool
            # work through the exp->mask chain.
            def qk_steps(cc8, t5):
                src = wqt if cc8 < 4 else wkt
                cbase = (cc8 % 4) * P
                acc = ps.tile([P, 512], f32, tag="gacc", bufs=2)
                for dc in range(DC):
                    nc.tensor.matmul(
                        acc[:],
                        src[:, dc, cbase:cbase + P],
                        xt[:, dc, t5 * 512:(t5 + 1) * 512],
                        start=(dc == 0), stop=(dc == DC - 1),
                    )
                    if dc == DC - 1:
                        if qkt_act:
                            nc.scalar.copy(
                                qkt[:, cc8, t5 * 512:(t5 + 1) * 512], acc[:])
                        else:
                            nc.vector.tensor_copy(
                                out=qkt[:, cc8, t5 * 512:(t5 + 1) * 512],
                                in_=acc[:])
                    yield

            def v_steps(t8, evict="dve"):
                acc = ps.tile([P, 512], f32, tag="gacc", bufs=2)
                for dc in range(DC):
                    nc.tensor.matmul(
                        acc[:],
                        xt[:, dc, t8 * P:(t8 + 1) * P],
                        wvt[:, dc, :],
                        start=(dc == 0), stop=(dc == DC - 1),
                    )
                    if dc == DC - 1:
                        # filler v groups evict on ACT so the DVE queue's
                        # qkt evictions (which gate the next pair's scores)
                        # aren't delayed behind them (Pool cannot read
                        # PSUM on real hardware)
                        if evict == "act":
                            nc.scalar.copy(
                                vaug[:, t8, :, 0:hd],
                                acc[:].rearrange("p (h j) -> p h j", h=HPG))
                        else:
                            nc.vector.tensor_copy(
                                out=vaug[:, t8, :, 0:hd],
                                in_=acc[:].rearrange("p (h j) -> p h j",
                                                     h=HPG))
                    yield

            def proj_steps(t5, oc, evict="dve"):
                acc = ps.tile([P, 512], f32, tag="gacc", bufs=2)
                for cc in range(GC // P):
                    nc.tensor.matmul(
                        acc[:],
                        wpt[:, cc, oc * P:(oc + 1) * P],
                        outt[:, cc, t5 * 512:(t5 + 1) * 512],
                        start=(cc == 0), stop=(cc == GC // P - 1),
                    )
                    if cc == GC // P - 1:
                        # groups interleaved with attention evict on ACT so
                        # the DVE norm chains aren't queued behind them;
                        # the tail alternates ACT/DVE
                        yt = ytp.tile([P, 512], bf16, tag="yt")
                        if evict == "act":
                            nc.scalar.copy(yt[:], acc[:])
                        else:
                            nc.vector.tensor_copy(out=yt[:], in_=acc[:])
                        nc.sync.dma_start(
                            yT[oc * P:(oc + 1) * P,
                               t5 * 512:(t5 + 1) * 512], yt[:])
                    yield

            class _Fillers:
                """FIFO of group generators; take(n) emits up to n PE
                matmuls by advancing the head generator. A partially
                advanced group holds a gacc PSUM ring slot, so direct
                (non-filler) gacc groups must finish_open() first or the
                2-slot ring can wrap onto the open group and deadlock the
                in-order PE queue."""

                def __init__(self):
                    self.q = deque()
                    self.open = False

                def add(self, gen):
                    self.q.append(gen)
                    return gen

                def take(self, n):
                    while n > 0 and self.q:
                        try:
                            next(self.q[0])
                            self.open = True
                            n -= 1
                        except StopIteration:
                            self.q.popleft()
                            self.open = False

                def finish_open(self):
                    if self.q and self.open:
                        g = self.q.popleft()
                        for _ in g:
                            pass
                        self.open = False

                def finish_through(self, gen):
                    # advance the FIFO until `gen` has fully emitted: a
                    # consumer is about to be emitted whose semaphores only
                    # exist if the producer's instructions precede it
                    while gen in self.q:
                        try:
                            next(self.q[0])
                            self.open = True
                        except StopIteration:
                            self.q.popleft()
                            self.open = False

                def drain(self):
                    self.finish_open()
                    while self.q:
                        g = self.q.popleft()
                        for _ in g:
                            pass

            fl = _Fillers()

            def run_now(gen):
                fl.finish_open()
                for _ in gen:
                    pass

            # ---- causal attention, head-pair processed, transposed ----
            def attn_pair(hp, qc, norm_pool=False):
                qA = qkt[0:hd, hp, :]
                qB = qkt[hd:P, hp, :]
                kA = qkt[0:hd, 4 + hp, :]
                kB = qkt[hd:P, 4 + hp, :]
                accp = ps.tile([P, 2, 512], f32, tag="accp", bufs=1)
                # qc=0: [0..3] (kb=0 is the full-width diagonal block);
                # qc=1: full-width below-diagonal blocks first, then the
                # diagonal / partial-width blocks
                order = list(range(4 * qc)) + [4 * qc + p for p in range(4)]
                nblk = len(order)
                batches = [order[i:i + blk_batch]
                           for i in range(0, nblk, blk_batch)]

                def scores(batch):
                    res = []
                    for kb in batch:
                        p_off = kb - 4 * qc    # >=0 on diagonal blocks
                        start_col = max(0, p_off) * P
                        width = 512 - start_col
                        sp2 = ps.tile([P, 2, 512], f32, tag="sp2", bufs=2)
                        nc.tensor.matmul(
                            sp2[0:P, 0, 0:width],
                            kA[:, kb * P:(kb + 1) * P],
                            qA[:, qc * 512 + start_col:(qc + 1) * 512],
                            start=True, stop=True,
                        )
                        nc.tensor.matmul(
                            sp2[0:P, 1, 0:width],
                            kB[:, kb * P:(kb + 1) * P],
                            qB[:, qc * 512 + start_col:(qc + 1) * 512],
                            start=True, stop=True,
                        )
                        res.append((kb, start_col, width, sp2))
                    return res

                def expmask(sps):
                    res = []
                    for kb, start_col, width, sp2 in sps:
                        pt2 = ptp.tile([P, 2, 512], bf16, tag="pt2")
                        nc.scalar.activation(pt2[:, :, 0:width],
                                             sp2[:, :, 0:width],
                                             Exp, scale=scale)
                        if kb - 4 * qc >= 0:   # triangular mask part
                            eng = nc.gpsimd if mask_pool else nc.vector
                            eng.tensor_tensor(
                                pt2[:, :, 0:P], pt2[:, :, 0:P],
                                tri[:].rearrange("p (o k) -> p o k", o=1)
                                      .to_broadcast([P, 2, P]), mult)
                        res.append((kb, start_col, width, pt2))
                    return res

                def pv(pts, i0):
                    for j, (kb, start_col, width, pt2) in enumerate(pts):
                        i = i0 + j
                        nc.tensor.matmul(
                            accp[:, 0, start_col:512],
                            vaug[:, kb, 2 * hp, :],
                            pt2[:, 0, 0:width],
                            start=(i == 0), stop=(i == nblk - 1),
                        )
                        nc.tensor.matmul(
                            accp[:, 1, start_col:512],
                            vaug[:, kb, 2 * hp + 1, :],
                            pt2[:, 1, 0:width],
                            start=(i == 0), stop=(i == nblk - 1),
                        )

                # lookahead-1 software pipeline: the PE queue runs
                # [s(b0) s(b1) fillers pv(b0) s(b2) fillers pv(b1) ...] so
                # the PE streams scores of batch j+1 (plus fillers) while
                # ACT/Pool run exp+mask of batch j. sp2's 2-slot ring
                # makes s(b_{j+1}) wait exactly until exp(b_j) has read.
                # Batches with masked (diagonal) blocks have the longer
                # exp->mask chain, so they pull more fillers.
                fl.take(pre_fill)
                sps = scores(batches[0])
                i0 = 0
                for bi in range(len(batches)):
                    pts = expmask(sps)
                    nxt = (scores(batches[bi + 1])
                           if bi + 1 < len(batches) else None)
                    masked = any(kb - 4 * qc >= 0 for kb, _, _, _ in pts)
                    fl.take(fill if masked else 1)
                    pv(pts, i0)
                    i0 += len(pts)
                    sps = nxt

                # normalize: psum rows 64..127 hold rowsum replicated 64x
                # (from vaug's ones block). ONE full-accp copy to SBUF --
                # same DVE time as copying only the sums (the free-dim size
                # per lane is identical) -- releases the PSUM accumulator
                # immediately, so the next pair's pv never waits on the
                # reciprocal+multiply tail. That tail (fast reciprocal on
                # DVE -- the custom op misreads PSUM, hence SBUF staging --
                # and per-head multiplies on Pool/DVE) runs off the
                # critical path.
                accs = small.tile([P, 2, 512], f32, tag="accs", bufs=2)
                nc.vector.tensor_copy(out=accs[:], in_=accp[:])
                rsb = small.tile([hd, 2, 512], f32, tag="rsb")
                if safe_recip:
                    # stage the row-sums to a base-0 tile before the custom
                    # DVE op (off the critical path; accp already released)
                    rss = small.tile([hd, 2, 512], f32, tag="rss")
                    nc.vector.tensor_copy(out=rss[:], in_=accs[hd:P, :, :])
                    nc.vector.reciprocal_approx_fast(out=rsb[:], in_=rss[:])
                else:
                    nc.vector.reciprocal_approx_fast(out=rsb[:],
                                                     in_=accs[hd:P, :, :])
                meng = nc.gpsimd if norm_pool else nc.vector
                for h01 in (0, 1):
                    prow = hd * h01
                    meng.tensor_tensor(
                        outt[prow:prow + hd, hp, qc * 512:(qc + 1) * 512],
                        accs[0:hd, h01, :], rsb[:, h01, :], mult)

            # ---- emission schedule ----
            # The exp stream on ACT (~34us/iter) must be spread across the
            # whole program: every attention span needs enough independent
            # PE ballast alongside it. Phase 1 (qc=0, exp-light) keeps only
            # v(4..7) as fillers; the t5=1 q/k projections move into phase
            # 2 (qc=1, exp-heavy), staggered one pair ahead of their
            # consumer, together with the token-half-0 output projections.
            # proj(0)'s leftovers cover the final norm chains; the
            # token-half-1 projections close the iteration.
            for t8 in range(4):
                run_now(v_steps(t8))
            for hp in range(GC // P):
                run_now(qk_steps(hp, 0))
                run_now(qk_steps(4 + hp, 0))
                if hp == 0:
                    for t8 in range(4, S // P):
                        fl.add(v_steps(t8, evict="act"))
                attn_pair(hp, 0)
            fl.drain()
            run_now(qk_steps(0, 1))
            run_now(qk_steps(4, 1))
            qk_pend = {}
            for hp in range(GC // P):
                if hp + 1 < GC // P:
                    qk_pend[hp + 1] = (fl.add(qk_steps(hp + 1, 1)),
                                       fl.add(qk_steps(4 + hp + 1, 1)))
                fl.add(proj_steps(0, 2 * hp, evict="act"))
                if hp in qk_pend:
                    for g in qk_pend.pop(hp):
                        fl.finish_through(g)
                attn_pair(hp, 1, norm_pool=pool_mults)
            fl.drain()
            for oc in (1, 3, 5, 7):
                run_now(proj_steps(0, oc))
            for oc in range(D // P):
                run_now(proj_steps(1, oc, evict="act" if oc % 2 == 0
                                          else "dve"))

    nc.compile()
    return nc


def _get_nc(repeat=1, **kw):
    key = ("nc", repeat, tuple(sorted(kw.items())))
    if key not in _CACHE:
        _CACHE[key] = _build(repeat, **kw)
    return _CACHE[key]


def _bf16(a):
    from ml_dtypes import bfloat16
    return np.ascontiguousarray(a.astype(bfloat16))


def make_in_maps(x, w_attn, w_proj):
    """Per-core input shards (core c -> batch c//2, head-group c%2)."""
    in_maps = []
    xTs = [_bf16(x[b].T) for b in range(B)]
    wqs = [_bf16(w_attn[:, g * GC:(g + 1) * GC]) for g in range(2)]
    wks = [_bf16(w_attn[:, D + g * GC:D + (g + 1) * GC]) for g in range(2)]
    wvs = [_bf16(w_attn[:, 2 * D + g * GC:2 * D + (g + 1) * GC])
           for g in range(2)]
    wps = [_bf16(w_proj[g * GC:(g + 1) * GC, :]) for g in range(2)]
    for c in range(8):
        b, g = divmod(c, 2)
        in_maps.append({
            "xT": xTs[b],
            "wq": wqs[g],
            "wk": wks[g],
            "wv": wvs[g],
            "wp": wps[g],
        })
    return in_maps


def kernel(x, w_attn, b_attn, w_proj, b_proj):
    x = np.asarray(x, dtype=np.float32)
    w_attn = np.asarray(w_attn, dtype=np.float32)
    b_attn = np.asarray(b_attn, dtype=np.float32)
    w_proj = np.asarray(w_proj, dtype=np.float32)
    b_proj = np.asarray(b_proj, dtype=np.float32)

    if np.any(b_attn):
        # Spec guarantees b_attn == 0 (fill: zeros); exact fallback if not.
        return _numpy_reference(x, w_attn, b_attn, w_proj, b_proj)

    in_maps = make_in_maps(x, w_attn, w_proj)
    results = _run_cached(in_maps)
    y = np.empty((B, S, D), np.float32)
    for b in range(B):
        y[b] = (results[2 * b]["yT"].astype(np.float32).T
                + results[2 * b + 1]["yT"].astype(np.float32).T + b_proj)
    return y


def _run_cached(in_maps):
    """Execute the compiled module on 8 cores; the jitted PJRT runner is
    built once and reused so repeated kernel() calls skip retracing."""
    import jax
    from jax.sharding import Mesh, NamedSharding, PartitionSpec
    from jax.experimental.shard_map import shard_map
    import concourse.mybir as mybir
    from concourse.bass2jax import (_bass_exec_p, install_neuronx_cc_hook,
                                    partition_id_tensor)

    if "runner" not in _CACHE:
        install_neuronx_cc_hook()
        nc = _get_nc()
        partition_name = (nc.partition_id_tensor.name
                          if nc.partition_id_tensor else None)
        in_names, out_names, out_avals, zero_outs = [], [], [], []
        for alloc in nc.m.functions[0].allocations:
            if not isinstance(alloc, mybir.MemoryLocationSet):
                continue
            name = alloc.memorylocations[0].name
            if alloc.kind == "ExternalInput":
                if name != partition_name:
                    in_names.append(name)
            elif alloc.kind == "ExternalOutput":
                shape = tuple(alloc.tensor_shape)
                dtype = mybir.dt.np(alloc.dtype)
                out_names.append(name)
                out_avals.append(jax.core.ShapedArray(shape, dtype))
                zero_outs.append(np.zeros((8 * shape[0], *shape[1:]), dtype))
        all_in_names = list(in_names) + list(out_names)
        if partition_name is not None:
            all_in_names.append(partition_name)

        def _body(*args):
            operands = list(args)
            if partition_name is not None:
                operands.append(partition_id_tensor())
            return tuple(_bass_exec_p.bind(
                *operands,
                out_avals=tuple(out_avals),
                in_names=tuple(all_in_names),
                out_names=tuple(out_names),
                lowering_input_output_aliases=(),
                sim_require_finite=True,
                sim_require_nnan=True,
                nc=nc,
            ))

        devices = jax.devices()[:8]
        mesh = Mesh(np.asarray(devices), ("core",))
        n_ops = len(in_names) + len(out_names)
        fn = jax.jit(shard_map(
            _body, mesh=mesh,
            in_specs=(PartitionSpec("core"),) * n_ops,
            out_specs=(PartitionSpec("core"),) * len(out_names),
            check_rep=False), keep_unused=True)
        shard = NamedSharding(mesh, PartitionSpec("core"))
        zeros_dev = [jax.device_put(z, shard) for z in zero_outs]
        _CACHE["runner"] = (fn, in_names, out_names, zeros_dev, shard)

    fn, in_names, out_names, zeros_dev, shard = _CACHE["runner"]
    import jax
    concat_in = [np.concatenate([np.asarray(in_maps[c][n]) for c in range(8)],
                                axis=0) for n in in_names]
    dev_in = [jax.device_put(a, shard) for a in concat_in]
    out_arrs = fn(*dev_in, *zeros_dev)
    results = []
    for c in range(8):
        results.append({
            name: np.asarray(out_arrs[i]).reshape(8, -1, 1024)[c]
            for i, name in enumerate(out_names)})
    return results


def _numpy_reference(x, w_attn, b_attn, w_proj, b_proj):
    qkv = x @ w_attn + b_attn
    q, k, v = np.split(qkv, 3, axis=-1)

    def heads(t):
        return t.reshape(B, S, H, hd).transpose(0, 2, 1, 3)

    q, k, v = heads(q), heads(k), heads(v)
    scores = np.einsum("bhqd,bhkd->bhqk", q, k) / np.sqrt(np.float32(hd))
    causal = np.tril(np.ones((S, S), dtype=bool))[None, None]
    scores = np.where(causal, scores, -1e9)
    scores -= scores.max(axis=-1, keepdims=True)
    attn = np.exp(scores)
    attn /= attn.sum(axis=-1, keepdims=True)
    out = np.einsum("bhqk,bhkd->bhqd", attn, v)
    out = out.transpose(0, 2, 1, 3).reshape(B, S, D)
    return out @ w_proj + b_proj

